# revision 36
# baseline (speedup 1.0000x reference)
"""Trainium2 Bass kernel for ContrastiveVideoAudioSimilarity.

Math (per batch element b, fully folded form):
  q        = probe @ wq.T + bq                      # [1024] -> heads [16, 64]
  ck[h,:]  = q[h] @ wk[h*64:(h+1)*64, :]            # [16, 1024]  (host folded)
  scores   = x @ ck.T / 8                           # [T*S, 16]; bk shift cancels in softmax
  attn     = softmax over S
  cx[t,h]  = sum_s attn[s,h] * x[t,s,:]             # [T, 16, 1024]
  ctx[t,h*64+d] = cx[t,h] @ wv[h*64+d,:] + bv       # per-head V proj of pooled vector
  pooled   = ctx @ wo.T + bo ; LayerNorm ; proj ; L2-normalize both sides; scaled dot.

Sharding: batch B=8, one batch element per NeuronCore (8 cores), params replicated.
Host precomputes folded/transposed weights (CKT, wvT, woT, projT) so the device
never touches wq/wk and needs no on-device weight transposes.
"""

import sys

for _p in ("/opt/trn_rl_repo", "/root/.axon_site/_ro/trn_rl_repo"):
    if _p not in sys.path:
        sys.path.insert(0, _p)

import numpy as np
import ml_dtypes

import concourse.bass as bass
import concourse.tile as tile
from concourse import bacc, mybir
from concourse.masks import make_identity

F32 = mybir.dt.float32
F32R = mybir.dt.float32r
BF16 = mybir.dt.bfloat16

B, T, S, DV, DA, NH, DH, L2 = 8, 32, 256, 1024, 512, 16, 64, 2048
EPS = 1e-6
FPG = 4  # frames per group


def build_nc(n_groups=T // FPG):
    """Build the per-core Bass program. n_groups*FPG = number of frames processed."""
    nT = n_groups * FPG  # frames
    nc = bacc.Bacc("TRN2", target_bir_lowering=False, debug=False)

    xv = nc.dram_tensor("xv", [nT * S, DV], F32, kind="ExternalInput").ap()
    aud = nc.dram_tensor("aud", [L2, DA], F32, kind="ExternalInput").ap()
    cktb = nc.dram_tensor("cktb", [128, 256], BF16, kind="ExternalInput").ap()
    wvt = nc.dram_tensor("wvt", [128, 8, DV], BF16, kind="ExternalInput").ap()
    bvp = nc.dram_tensor("bvp", [128, 8], F32, kind="ExternalInput").ap()
    wot = nc.dram_tensor("wot", [128, 8, DV], BF16, kind="ExternalInput").ap()
    bo2 = nc.dram_tensor("bo2", [1, DV], F32, kind="ExternalInput").ap()
    pjt = nc.dram_tensor("pjt", [128, 8, DA], BF16, kind="ExternalInput").ap()
    pjb = nc.dram_tensor("pjb", [1, DA], F32, kind="ExternalInput").ap()
    sca = nc.dram_tensor("sca", [1, 2], F32, kind="ExternalInput").ap()
    out = nc.dram_tensor("out", [nT, L2], F32, kind="ExternalOutput").ap()

    with tile.TileContext(nc) as tc:
        with (
            tc.tile_pool(name="const", bufs=1) as constp,
            tc.tile_pool(name="persist", bufs=1) as persist,
            tc.tile_pool(name="dram", bufs=1, space="DRAM") as dramp,
        ):
            # ---- constants / params resident in SBUF ----
            idb = constp.tile([128, 128], BF16)
            make_identity(nc, idb)
            idf = constp.tile([128, 128], F32)
            make_identity(nc, idf)
            ck_s = constp.tile([128, 256], BF16)
            nc.gpsimd.dma_start(ck_s[:], cktb)
            ck_v = ck_s.rearrange("p (co m) -> p co m", co=8)
            # weight/bias tiles allocated now, loaded mid-stage-1 (they're not
            # needed before stage 2, and loading them up-front delays group 0)
            wvt_s = persist.tile([128, 8, DV], BF16)
            bvp_s = constp.tile([128, 8], F32)
            wot_s = persist.tile([128, 8, DV], BF16)
            pjt_s = persist.tile([128, 8, DA], BF16)
            bo2_s = constp.tile([nT, DV], F32)
            pjb_s = constp.tile([nT, DA], F32)
            sca_s = constp.tile([nT, 2], F32)

            def bcast_dma(t_, src, parts):
                """Load a [1, free] DRAM row broadcast across `parts` partitions."""
                src_b = bass.AP(
                    tensor=src.tensor, offset=src.offset,
                    ap=[[0, parts]] + list(src.ap[1:]),
                )
                nc.gpsimd.dma_start(out=t_[:], in_=src_b)

            def emit_weight_loads_a():
                nc.gpsimd.dma_start(wvt_s[:], wvt)
                nc.gpsimd.dma_start(bvp_s[:], bvp)

            def emit_weight_loads_b():
                nc.gpsimd.dma_start(wot_s[:], wot)
                bcast_dma(bo2_s, bo2, nT)

            def emit_weight_loads_c():
                nc.gpsimd.dma_start(pjt_s[:], pjt)
                bcast_dma(pjb_s, pjb, nT)
                bcast_dma(sca_s, sca, nT)

            epsb = constp.tile([128, 1], F32)
            nc.vector.memset(epsb, EPS)

            # cx^T accumulator: cxt[ci, co, t, h] = cx[t, h, co*128+ci]
            cxt = persist.tile([128, 8, nT, NH], BF16)

            # audio tiles (filled after stage 1 is issued, so the video DMAs
            # are first in the gpsimd issue queue)
            audtb = persist.tile([128, 16, 4, 128], BF16)  # [d_in, lblk, d_out, l_in]
            rl_b = constp.tile([T, L2], F32)  # 1/||audio_l|| bcast on partitions

            audb = persist.tile([128, 16, DA], BF16)  # [l_in, l_blk, d]

            def emit_audio_load(audp):
                """Audio load + norm stats; interleaved mid-stage-1.

                (audtb transposes happen later on TensorE, after stage 2.)
                """
                nc.gpsimd.dma_start(
                    out=audb[:], in_=aud.rearrange("(b a) d -> a b d", a=128)
                )
                ast = audp.tile([128, 16, 6], F32)
                amv = audp.tile([128, 16, 2], F32)
                for b in range(16):
                    nc.vector.bn_stats(out=ast[:, b, :], in_=audb[:, b, :])
                    nc.vector.bn_aggr(out=amv[:, b, :], in_=ast[:, b, :])
                ssum = audp.tile([128, 16], F32)
                nc.vector.tensor_tensor(
                    out=ssum[:], in0=amv[:, :, 0], in1=amv[:, :, 0],
                    op=mybir.AluOpType.mult,
                )
                nc.vector.tensor_add(ssum[:], ssum[:], amv[:, :, 1])
                # ||a_l|| = sqrt(DA * (var + mu^2))
                nc.scalar.activation(
                    out=ssum[:], in_=ssum[:],
                    func=mybir.ActivationFunctionType.Sqrt, scale=float(DA),
                )
                rml = audp.tile([128, 16], F32)
                nc.vector.reciprocal(out=rml[:], in_=ssum[:])
                # flat l-order roundtrip via DRAM: rl_flat[b*128+a] = rml[a, b]
                rld = dramp.tile([L2], F32)
                nc.scalar.dma_start(
                    out=rld.rearrange("(b a) -> a b", a=128), in_=rml[:]
                )
                nc.gpsimd.dma_start(
                    out=rl_b[:nT],
                    in_=bass.AP(tensor=rld.tensor, offset=rld.offset,
                                ap=[[0, nT]] + list(rld.ap)),
                )

            # ctx^T[e, t] (stage-2 output, filled per half)
            ctxt = persist.tile([128, 8, nT], BF16)
            HT = nT // 2  # frames per tail-half

            def emit_audio_transposes(ps234):
                """audb [l, b, d] -> audtb [d, b, do, l] via TensorE."""
                for b in range(16):
                    au_ps = ps234.tile([128, 4, 128], BF16, tag="tr")
                    for do in range(4):
                        nc.tensor.transpose(
                            au_ps[:, do, :],
                            audb[:, b, do * 128:(do + 1) * 128],
                            idb[:],
                        )
                    if b % 2 == 0:
                        nc.vector.tensor_copy(out=audtb[:, b, :, :], in_=au_ps[:])
                    else:
                        nc.scalar.activation(
                            out=audtb[:, b, :, :], in_=au_ps[:],
                            func=mybir.ActivationFunctionType.Copy,
                        )

            half_state = {}

            def emit_stage23(half, s3, ps234, mmb=2, trb=1):
                """V-proj + wo/LN + audio proj for HT frames."""
                t0 = half * HT
                ts_ = slice(t0, t0 + HT)
                # stage 2: per-head V-projection ctx^T[e, t]
                for eo in range(8):
                    ctp = ps234.tile([128, HT], F32, tag="mm", bufs=mmb)
                    for hh in range(2):
                        h = 2 * eo + hh
                        for co in range(8):
                            nc.tensor.matmul(
                                ctp[64 * hh:64 * (hh + 1), :],
                                wvt_s[:, co, h * DH:(h + 1) * DH],
                                cxt[:, co, ts_, h],
                                start=(co == 0),
                                stop=(co == 7),
                                skip_group_check=True,
                            )
                    nc.vector.tensor_scalar_add(
                        out=ctxt[:, eo, ts_], in0=ctp[:], scalar1=bvp_s[:, eo:eo + 1]
                    )
                # stage 3: wo projection + LayerNorm
                pooled = s3.tile([HT, DV], F32, tag=f"pl{half}")
                for n in range(2):
                    pp = ps234.tile([HT, 512], F32, tag="mm", bufs=mmb)
                    for eo in range(8):
                        nc.tensor.matmul(
                            pp[:],
                            ctxt[:, eo, ts_],
                            wot_s[:, eo, 512 * n:512 * (n + 1)],
                            start=(eo == 0),
                            stop=(eo == 7),
                        )
                    nc.vector.tensor_tensor(
                        out=pooled[:, 512 * n:512 * (n + 1)], in0=pp[:],
                        in1=bo2_s[:HT, 512 * n:512 * (n + 1)],
                        op=mybir.AluOpType.add,
                    )
                lst = s3.tile([HT, 2, 6], F32, tag=f"ls{half}")
                nc.vector.bn_stats(out=lst[:, 0, :], in_=pooled[:, 0:512])
                nc.vector.bn_stats(out=lst[:, 1, :], in_=pooled[:, 512:1024])
                lmv = s3.tile([HT, 2], F32, tag=f"lm{half}")
                nc.vector.bn_aggr(out=lmv[:], in_=lst[:])
                sd = s3.tile([HT, 1], F32, tag=f"sd{half}")
                nc.scalar.activation(
                    out=sd[:], in_=lmv[:, 1:2],
                    func=mybir.ActivationFunctionType.Sqrt, bias=epsb[:HT],
                )
                rstd = s3.tile([HT, 1], F32, tag=f"rs{half}")
                nc.vector.reciprocal(out=rstd[:], in_=sd[:])
                # ln_g/ln_b are folded into pjt/pjb host-side; only z-score here
                nc.vector.tensor_scalar(
                    out=pooled[:], in0=pooled[:],
                    scalar1=lmv[:, 0:1], scalar2=rstd[:],
                    op0=mybir.AluOpType.subtract, op1=mybir.AluOpType.mult,
                )
                # transpose pooled -> [f, t]
                plt = s3.tile([128, 8, HT], BF16, tag=f"pt{half}")
                for fo in range(8):
                    ptp = ps234.tile([128, HT], F32, tag="tr", bufs=trb)
                    nc.tensor.transpose(
                        ptp[:], pooled[:, 128 * fo:128 * (fo + 1)], idf[:HT, :HT]
                    )
                    nc.vector.tensor_copy(out=plt[:, fo, :], in_=ptp[:])
                # audio-dim projection
                vtp = ps234.tile([HT, DA], F32, tag="mm", bufs=mmb)
                for fo in range(8):
                    nc.tensor.matmul(
                        vtp[:],
                        plt[:, fo, :],
                        pjt_s[:, fo, :],
                        start=(fo == 0),
                        stop=(fo == 7),
                    )
                vt = s3.tile([HT, DA], F32, tag=f"vt{half}")
                nc.vector.tensor_tensor(
                    out=vt[:], in0=vtp[:], in1=pjb_s[:HT, :],
                    op=mybir.AluOpType.add,
                )
                # s_t = exp(logit_scale) / ||vt||
                vst = s3.tile([HT, 6], F32, tag=f"vs{half}")
                nc.vector.bn_stats(out=vst[:], in_=vt[:])
                vmv = s3.tile([HT, 2], F32, tag=f"vm{half}")
                nc.vector.bn_aggr(out=vmv[:], in_=vst[:])
                vss = s3.tile([HT, 1], F32, tag=f"vq{half}")
                nc.vector.tensor_tensor(
                    out=vss[:], in0=vmv[:, 0:1], in1=vmv[:, 0:1],
                    op=mybir.AluOpType.mult,
                )
                nc.vector.tensor_add(vss[:], vss[:], vmv[:, 1:2])
                nc.scalar.activation(
                    out=vss[:], in_=vss[:],
                    func=mybir.ActivationFunctionType.Sqrt, scale=float(DA),
                )
                st = s3.tile([HT, 1], F32, tag=f"st{half}")
                nc.vector.reciprocal(out=st[:], in_=vss[:])
                nc.vector.tensor_scalar_mul(
                    out=st[:], in0=st[:], scalar1=sca_s[:HT, 0:1]
                )
                # vt^T as bf16 for the similarity matmul
                vttb = s3.tile([128, 4, HT], BF16, tag=f"vb{half}")
                for do in range(4):
                    vtp2 = ps234.tile([128, HT], F32, tag="tr", bufs=trb)
                    nc.tensor.transpose(
                        vtp2[:], vt[:, 128 * do:128 * (do + 1)], idf[:HT, :HT]
                    )
                    nc.vector.tensor_copy(out=vttb[:, do, :], in_=vtp2[:])
                half_state[half] = (vttb, st)

            def emit_sim(half, s3, ps234, mmb=2):
                """Stage 4: similarity vs all audio tokens."""
                t0 = half * HT
                ts_ = slice(t0, t0 + HT)
                vttb, st = half_state[half]
                for lc in range(4):
                    smp = ps234.tile([HT, 512], F32, tag="mm", bufs=mmb)
                    for do in range(4):
                        nc.tensor.matmul(
                            smp[:],
                            vttb[:, do, :],
                            audtb[:, 4 * lc:4 * (lc + 1), do, :],
                            start=(do == 0),
                            stop=(do == 3),
                        )
                    o1 = s3.tile([HT, 512], F32, tag=f"o1{half}", bufs=2)
                    nc.vector.tensor_tensor(
                        out=o1[:], in0=smp[:],
                        in1=rl_b[:HT, 512 * lc:512 * (lc + 1)],
                        op=mybir.AluOpType.mult,
                    )
                    nc.vector.tensor_scalar(
                        out=o1[:], in0=o1[:],
                        scalar1=st[:], scalar2=sca_s[:HT, 1:2],
                        op0=mybir.AluOpType.mult, op1=mybir.AluOpType.add,
                    )
                    nc.scalar.dma_start(out=out[ts_, 512 * lc:512 * (lc + 1)], in_=o1[:])

            # ---- stage 1: per-frame attention pooling in x-space, with the
            # tail stages interleaved to fill DMA-bound PE gaps ----
            with (
                tc.tile_pool(name="xb", bufs=3) as xbp,
                tc.tile_pool(name="xt", bufs=2) as xtp,
                tc.tile_pool(name="sm", bufs=2) as smp_,
                tc.tile_pool(name="at", bufs=2) as atp_,
                tc.tile_pool(name="audp", bufs=1) as audp,
                tc.tile_pool(name="s3", bufs=1) as s3,
                tc.tile_pool(name="ps_xt", bufs=3, space="PSUM") as ps_xt,
                tc.tile_pool(name="ps_sc", bufs=1, space="PSUM") as ps_sc,
                tc.tile_pool(name="ps_cx", bufs=1, space="PSUM") as ps_cx,
                tc.tile_pool(name="ps234", bufs=1, space="PSUM") as ps234,
            ):
                def emit_attn_cx(g, xbg, attn_t):
                    """Attn transpose + cx for group g (emitted one group late
                    so PE isn't head-of-line blocked on g's softmax)."""
                    attn_s = atp_.tile([128, FPG, 2, NH], BF16)
                    for f in range(FPG):
                        for so in range(2):
                            atp = ps_xt.tile([128, NH], BF16, tag="xt")
                            nc.tensor.transpose(
                                atp[:],
                                attn_t[32 * f:32 * f + 16, 128 * so:128 * (so + 1)],
                                idb[32 * f:32 * f + 16, 32 * f:32 * f + 16],
                                tile_position=(32 * f, 0),
                            )
                            nc.vector.tensor_copy(out=attn_s[:, f, so, :], in_=atp[:])
                    # cx^T[c, h] = sum_s xb[s, c] * attn[s, h]
                    for f in range(FPG):
                        cxp = ps_cx.tile([128, 128], F32)
                        for co in range(8):
                            for so in range(2):
                                nc.tensor.matmul(
                                    cxp[:, co * NH:(co + 1) * NH],
                                    xbg[:, f, so, co * 128:(co + 1) * 128],
                                    attn_s[:, f, so, :],
                                    start=(so == 0),
                                    stop=(so == 1),
                                )
                        t = g * FPG + f
                        nc.vector.tensor_copy(
                            out=cxt[:, :, t, :],
                            in_=cxp.rearrange("p (co h) -> p co h", co=8),
                        )

                nxc = 0
                pend = None
                for g in range(n_groups):
                    if g == 2:
                        emit_weight_loads_a()
                    if g == 4:
                        emit_weight_loads_b()
                        emit_audio_load(audp)
                    if g == 5:
                        emit_weight_loads_c()
                    # load FPG frames, cast f32 -> bf16 in DMA; one DMA per
                    # frame so transposes can chase the load frame-by-frame
                    xbg = xbp.tile([128, FPG, 2, DV], BF16)  # [si, f, so, c]
                    for f in range(FPG):
                        base = (g * FPG + f) * S
                        nc.gpsimd.dma_start(
                            out=xbg[:, f, :, :],
                            in_=xv[base:base + S, :].rearrange(
                                "(so si) c -> si so c", so=2
                            ),
                        )
                    # transpose to [ci, f, so, co, si] via TensorE (PE identity
                    # matmul); 8 blocks batched into one PSUM bank, one wide
                    # PSUM->SBUF copy per (f, so), alternating vector/gpsimd.
                    xtg = xtp.tile([128, FPG, 2, 8, 128], BF16)
                    for f in range(FPG):
                        for so in range(2):
                            xt_ps = ps_xt.tile([128, 8, 128], BF16, tag="xt")
                            for co in range(8):
                                nc.tensor.transpose(
                                    xt_ps[:, co, :],
                                    xbg[:, f, so, co * 128:(co + 1) * 128],
                                    idb[:],
                                )
                            if nxc % 2 == 0:
                                nc.vector.tensor_copy(
                                    out=xtg[:, f, so, :, :], in_=xt_ps[:]
                                )
                            else:
                                nc.scalar.activation(
                                    out=xtg[:, f, so, :, :], in_=xt_ps[:],
                                    func=mybir.ActivationFunctionType.Copy,
                                )
                            nxc += 1
                    # scores^T: [16@32f, s=256] per frame, packed on partitions.
                    # ck is zero-padded to M=32 so every partition row of scp is
                    # written (junk-but-finite rows 16..32 of each block).
                    scp = ps_sc.tile([128, S], F32)
                    for f in range(FPG):
                        for co in range(8):
                            nc.tensor.matmul(
                                scp[32 * f:32 * f + 32, :],
                                ck_v[:, co, :],
                                xtg[:, f, :, co, :],
                                start=(co == 0),
                                stop=(co == 7),
                                tile_position=(0, 32 * f),
                            )
                    # softmax over s (free dim), whole group at once (junk rows
                    # stay finite: scores there are 0, exp(0-max)<=1)
                    negm = smp_.tile([128, 1], F32)
                    et = smp_.tile([128, S], F32)
                    esum = smp_.tile([128, 1], F32)
                    rsum = smp_.tile([128, 1], F32)
                    attn_t = atp_.tile([128, S], BF16)  # attn^T [h@32f, s]
                    nc.vector.reduce_max(
                        out=negm[:], in_=scp[:], axis=mybir.AxisListType.X,
                        negate=True,
                    )
                    nc.scalar.activation(
                        out=et[:], in_=scp[:],
                        func=mybir.ActivationFunctionType.Exp,
                        bias=negm[:], scale=1.0, accum_out=esum[:],
                    )
                    nc.vector.reciprocal(out=rsum[:], in_=esum[:])
                    nc.vector.tensor_scalar_mul(attn_t[:], et[:], rsum[:])
                    # previous group's attn transpose + cx (keeps PE fed while
                    # this group's softmax runs on vector/scalar)
                    if pend is not None:
                        emit_attn_cx(*pend)
                    pend = (g, xbg, attn_t)
                    if g == 5:
                        emit_stage23(0, s3, ps234)
                    if g == 6:
                        emit_audio_transposes(ps234)
                        emit_sim(0, s3, ps234)
                emit_attn_cx(*pend)

            # half 1 runs after stage-1 drains; fresh pools with deeper rings
            with (
                tc.tile_pool(name="s3b", bufs=1) as s3b,
                tc.tile_pool(name="ps_t2", bufs=1, space="PSUM") as ps_t2,
            ):
                emit_stage23(1, s3b, ps_t2, mmb=3, trb=3)
                emit_sim(1, s3b, ps_t2, mmb=3)

    nc.compile()
    return nc


def host_fold(probe, wq, wk, bq, wv, bv, wo, bo, ln_g, ln_b, proj_w, proj_b,
              logit_scale, logit_bias):
    """Fold weights on the host into device-friendly layouts."""
    f64 = np.float64
    qvec = probe.reshape(-1).astype(f64) @ wq.astype(f64).T + bq.astype(f64)
    q = qvec.reshape(NH, DH)
    ck = np.stack(
        [q[h] @ wk.astype(f64)[h * DH:(h + 1) * DH, :] for h in range(NH)]
    )  # [16, 1024]
    ck /= np.sqrt(f64(DH))
    # zero-pad heads to M=32 so the scores matmul writes full 32-row blocks
    ckp = np.zeros((32, DV), np.float64)
    ckp[:NH] = ck
    # cktb[ci, co*32+m] = ckp[m, co*128+ci]
    ckt = ckp.T.reshape(8, 128, 32).transpose(1, 0, 2).reshape(128, 256)
    cktb = ckt.astype(ml_dtypes.bfloat16)

    wvt = np.ascontiguousarray(
        wv.T.reshape(8, 128, DV).transpose(1, 0, 2)).astype(ml_dtypes.bfloat16)
    wot = np.ascontiguousarray(
        wo.T.reshape(8, 128, DV).transpose(1, 0, 2)).astype(ml_dtypes.bfloat16)
    # fold LayerNorm gain/bias into the audio projection:
    #   proj(LN_affine(z)) = z @ (proj_w * g)^T + (proj_b + proj_w @ b)
    pw = proj_w.astype(f64) * ln_g.astype(f64)[None, :]
    pb = proj_b.astype(f64) + proj_w.astype(f64) @ ln_b.astype(f64)
    pjt = np.ascontiguousarray(
        pw.T.reshape(8, 128, DA).transpose(1, 0, 2)).astype(ml_dtypes.bfloat16)
    bvp = np.ascontiguousarray(bv.reshape(8, 128).T).astype(np.float32)
    sca = np.array([[np.exp(np.float64(logit_scale[0])), logit_bias[0]]],
                   np.float32)
    return dict(
        cktb=np.ascontiguousarray(cktb),
        wvt=wvt, bvp=bvp, wot=wot,
        bo2=bo.reshape(1, DV).astype(np.float32),
        pjt=pjt, pjb=pb.reshape(1, DA).astype(np.float32),
        sca=sca,
    )


_NC_CACHE = {}


def kernel(video_x, audio_x, probe, wq, wk, wv, bq, bk, bv, wo, bo,
           ln_g, ln_b, proj_w, proj_b, logit_scale, logit_bias, T=None, H=None,
           W=None, **_unused):
    from concourse.bass_utils import run_bass_kernel_spmd

    video_x = np.asarray(video_x, np.float32)
    audio_x = np.asarray(audio_x, np.float32)
    params = host_fold(
        np.asarray(probe, np.float32), np.asarray(wq, np.float32),
        np.asarray(wk, np.float32), np.asarray(bq, np.float32),
        np.asarray(wv, np.float32), np.asarray(bv, np.float32),
        np.asarray(wo, np.float32), np.asarray(bo, np.float32),
        np.asarray(ln_g, np.float32), np.asarray(ln_b, np.float32),
        np.asarray(proj_w, np.float32), np.asarray(proj_b, np.float32),
        np.asarray(logit_scale, np.float32), np.asarray(logit_bias, np.float32),
    )
    if "nc" not in _NC_CACHE:
        _NC_CACHE["nc"] = build_nc()
    nc = _NC_CACHE["nc"]
    in_maps = []
    for b in range(B):
        m = dict(params)
        m["xv"] = np.ascontiguousarray(video_x[b])
        m["aud"] = np.ascontiguousarray(audio_x[b])
        in_maps.append(m)
    res = run_bass_kernel_spmd(nc, in_maps, core_ids=list(range(B)), trace=False)
    return np.stack([res.results[b]["out"] for b in range(B)], axis=0)



# revision 37
# speedup vs baseline: 1.0561x; 1.0561x over previous
"""Trainium2 Bass kernel for ContrastiveVideoAudioSimilarity.

Math (per batch element b, fully folded form):
  q        = probe @ wq.T + bq                      # [1024] -> heads [16, 64]
  ck[h,:]  = q[h] @ wk[h*64:(h+1)*64, :]            # [16, 1024]  (host folded)
  scores   = x @ ck.T / 8                           # [T*S, 16]; bk shift cancels in softmax
  attn     = softmax over S
  cx[t,h]  = sum_s attn[s,h] * x[t,s,:]             # [T, 16, 1024]
  ctx[t,h*64+d] = cx[t,h] @ wv[h*64+d,:] + bv       # per-head V proj of pooled vector
  pooled   = ctx @ wo.T + bo ; LayerNorm ; proj ; L2-normalize both sides; scaled dot.

Sharding: batch B=8, one batch element per NeuronCore (8 cores), params replicated.
Host precomputes folded/transposed weights (CKT, wvT, woT, projT) so the device
never touches wq/wk and needs no on-device weight transposes.
"""

import sys

for _p in ("/opt/trn_rl_repo", "/root/.axon_site/_ro/trn_rl_repo"):
    if _p not in sys.path:
        sys.path.insert(0, _p)

import numpy as np
import ml_dtypes

import concourse.bass as bass
import concourse.tile as tile
from concourse import bacc, mybir
from concourse.masks import make_identity

F32 = mybir.dt.float32
F32R = mybir.dt.float32r
BF16 = mybir.dt.bfloat16

B, T, S, DV, DA, NH, DH, L2 = 8, 32, 256, 1024, 512, 16, 64, 2048
EPS = 1e-6
FPG = 4  # frames per group


def build_nc(n_groups=T // FPG):
    """Build the per-core Bass program. n_groups*FPG = number of frames processed."""
    nT = n_groups * FPG  # frames
    nc = bacc.Bacc("TRN2", target_bir_lowering=False, debug=False)

    xv = nc.dram_tensor("xv", [nT * S, DV], F32, kind="ExternalInput").ap()
    aud = nc.dram_tensor("aud", [L2, DA], F32, kind="ExternalInput").ap()
    cktb = nc.dram_tensor("cktb", [128, 256], BF16, kind="ExternalInput").ap()
    wvt = nc.dram_tensor("wvt", [128, 8, DV], BF16, kind="ExternalInput").ap()
    bvp = nc.dram_tensor("bvp", [128, 8], F32, kind="ExternalInput").ap()
    wot = nc.dram_tensor("wot", [128, 8, DV], BF16, kind="ExternalInput").ap()
    bo2 = nc.dram_tensor("bo2", [1, DV], F32, kind="ExternalInput").ap()
    pjt = nc.dram_tensor("pjt", [128, 8, DA], BF16, kind="ExternalInput").ap()
    pjb = nc.dram_tensor("pjb", [1, DA], F32, kind="ExternalInput").ap()
    sca = nc.dram_tensor("sca", [1, 2], F32, kind="ExternalInput").ap()
    out = nc.dram_tensor("out", [nT, L2], F32, kind="ExternalOutput").ap()

    with tile.TileContext(nc) as tc:
        with (
            tc.tile_pool(name="const", bufs=1) as constp,
            tc.tile_pool(name="persist", bufs=1) as persist,
            tc.tile_pool(name="dram", bufs=1, space="DRAM") as dramp,
        ):
            # ---- constants / params resident in SBUF ----
            idb = constp.tile([128, 128], BF16)
            make_identity(nc, idb)
            idf = constp.tile([128, 128], F32)
            make_identity(nc, idf)
            ck_s = constp.tile([128, 256], BF16)
            nc.gpsimd.dma_start(ck_s[:], cktb)
            ck_v = ck_s.rearrange("p (co m) -> p co m", co=8)
            # weight/bias tiles allocated now, loaded mid-stage-1 (they're not
            # needed before stage 2, and loading them up-front delays group 0)
            wvt_s = persist.tile([128, 8, DV], BF16)
            bvp_s = constp.tile([128, 8], F32)
            wot_s = persist.tile([128, 8, DV], BF16)
            pjt_s = persist.tile([128, 8, DA], BF16)
            bo2_s = constp.tile([nT, DV], F32)
            pjb_s = constp.tile([nT, DA], F32)
            sca_s = constp.tile([nT, 2], F32)

            def bcast_dma(t_, src, parts):
                """Load a [1, free] DRAM row broadcast across `parts` partitions."""
                src_b = bass.AP(
                    tensor=src.tensor, offset=src.offset,
                    ap=[[0, parts]] + list(src.ap[1:]),
                )
                nc.gpsimd.dma_start(out=t_[:], in_=src_b)

            def emit_weight_loads_a():
                nc.gpsimd.dma_start(wvt_s[:], wvt)
                nc.gpsimd.dma_start(bvp_s[:], bvp)

            def emit_weight_loads_b():
                nc.gpsimd.dma_start(wot_s[:], wot)
                bcast_dma(bo2_s, bo2, nT)

            def emit_weight_loads_c():
                nc.gpsimd.dma_start(pjt_s[:], pjt)
                bcast_dma(pjb_s, pjb, nT)
                bcast_dma(sca_s, sca, nT)

            epsb = constp.tile([128, 1], F32)
            nc.vector.memset(epsb, EPS)

            # cx^T accumulator: cxt[ci, co, t, h] = cx[t, h, co*128+ci]
            cxt = persist.tile([128, 8, nT, NH], BF16)

            # audio tiles (filled after stage 1 is issued, so the video DMAs
            # are first in the gpsimd issue queue)
            audtb = persist.tile([128, 16, 4, 128], BF16)  # [d_in, lblk, d_out, l_in]
            rl_b = constp.tile([T, L2], F32)  # 1/||audio_l|| bcast on partitions

            audb = persist.tile([128, 16, DA], BF16)  # [l_in, l_blk, d]

            def emit_audio_load(audp):
                """Audio load + norm stats; interleaved mid-stage-1.

                (audtb transposes happen later on TensorE, after stage 2.)
                """
                nc.gpsimd.dma_start(
                    out=audb[:], in_=aud.rearrange("(b a) d -> a b d", a=128)
                )
                ast = audp.tile([128, 16, 6], F32)
                amv = audp.tile([128, 16, 2], F32)
                for b in range(16):
                    nc.vector.bn_stats(out=ast[:, b, :], in_=audb[:, b, :])
                    nc.vector.bn_aggr(out=amv[:, b, :], in_=ast[:, b, :])
                ssum = audp.tile([128, 16], F32)
                nc.vector.tensor_tensor(
                    out=ssum[:], in0=amv[:, :, 0], in1=amv[:, :, 0],
                    op=mybir.AluOpType.mult,
                )
                nc.vector.tensor_add(ssum[:], ssum[:], amv[:, :, 1])
                # ||a_l|| = sqrt(DA * (var + mu^2))
                nc.scalar.activation(
                    out=ssum[:], in_=ssum[:],
                    func=mybir.ActivationFunctionType.Sqrt, scale=float(DA),
                )
                rml = audp.tile([128, 16], F32)
                nc.vector.reciprocal(out=rml[:], in_=ssum[:])
                # flat l-order roundtrip via DRAM: rl_flat[b*128+a] = rml[a, b]
                rld = dramp.tile([L2], F32)
                nc.scalar.dma_start(
                    out=rld.rearrange("(b a) -> a b", a=128), in_=rml[:]
                )
                nc.gpsimd.dma_start(
                    out=rl_b[:nT],
                    in_=bass.AP(tensor=rld.tensor, offset=rld.offset,
                                ap=[[0, nT]] + list(rld.ap)),
                )

            # ctx^T[e, t] (stage-2 output, filled per half)
            ctxt = persist.tile([128, 8, nT], BF16)
            HT = nT // 2  # frames per tail-half

            def emit_audio_transposes(ps234):
                """audb [l, b, d] -> audtb [d, b, do, l] via TensorE."""
                for b in range(16):
                    au_ps = ps234.tile([128, 4, 128], BF16, tag="tr")
                    for do in range(4):
                        nc.tensor.transpose(
                            au_ps[:, do, :],
                            audb[:, b, do * 128:(do + 1) * 128],
                            idb[:],
                        )
                    if b % 2 == 0:
                        nc.vector.tensor_copy(out=audtb[:, b, :, :], in_=au_ps[:])
                    else:
                        nc.scalar.activation(
                            out=audtb[:, b, :, :], in_=au_ps[:],
                            func=mybir.ActivationFunctionType.Copy,
                        )

            half_state = {}

            def emit_stage23(half, s3, ps234, mmb=2, trb=1):
                """V-proj + wo/LN + audio proj for HT frames."""
                t0 = half * HT
                ts_ = slice(t0, t0 + HT)
                # stage 2: per-head V-projection ctx^T[e, t]
                for eo in range(8):
                    ctp = ps234.tile([128, HT], F32, tag="mm", bufs=mmb)
                    for hh in range(2):
                        h = 2 * eo + hh
                        for co in range(8):
                            nc.tensor.matmul(
                                ctp[64 * hh:64 * (hh + 1), :],
                                wvt_s[:, co, h * DH:(h + 1) * DH],
                                cxt[:, co, ts_, h],
                                start=(co == 0),
                                stop=(co == 7),
                                skip_group_check=True,
                            )
                    nc.vector.tensor_scalar_add(
                        out=ctxt[:, eo, ts_], in0=ctp[:], scalar1=bvp_s[:, eo:eo + 1]
                    )
                # stage 3: wo projection + LayerNorm
                pooled = s3.tile([HT, DV], F32, tag=f"pl{half}")
                for n in range(2):
                    pp = ps234.tile([HT, 512], F32, tag="mm", bufs=mmb)
                    for eo in range(8):
                        nc.tensor.matmul(
                            pp[:],
                            ctxt[:, eo, ts_],
                            wot_s[:, eo, 512 * n:512 * (n + 1)],
                            start=(eo == 0),
                            stop=(eo == 7),
                        )
                    nc.vector.tensor_tensor(
                        out=pooled[:, 512 * n:512 * (n + 1)], in0=pp[:],
                        in1=bo2_s[:HT, 512 * n:512 * (n + 1)],
                        op=mybir.AluOpType.add,
                    )
                lst = s3.tile([HT, 2, 6], F32, tag=f"ls{half}")
                nc.vector.bn_stats(out=lst[:, 0, :], in_=pooled[:, 0:512])
                nc.vector.bn_stats(out=lst[:, 1, :], in_=pooled[:, 512:1024])
                lmv = s3.tile([HT, 2], F32, tag=f"lm{half}")
                nc.vector.bn_aggr(out=lmv[:], in_=lst[:])
                sd = s3.tile([HT, 1], F32, tag=f"sd{half}")
                nc.scalar.activation(
                    out=sd[:], in_=lmv[:, 1:2],
                    func=mybir.ActivationFunctionType.Sqrt, bias=epsb[:HT],
                )
                rstd = s3.tile([HT, 1], F32, tag=f"rs{half}")
                nc.vector.reciprocal(out=rstd[:], in_=sd[:])
                # ln_g/ln_b are folded into pjt/pjb host-side; only z-score here
                nc.vector.tensor_scalar(
                    out=pooled[:], in0=pooled[:],
                    scalar1=lmv[:, 0:1], scalar2=rstd[:],
                    op0=mybir.AluOpType.subtract, op1=mybir.AluOpType.mult,
                )
                # transpose pooled -> [f, t]
                plt = s3.tile([128, 8, HT], BF16, tag=f"pt{half}")
                for fo in range(8):
                    ptp = ps234.tile([128, HT], F32, tag="tr", bufs=trb)
                    nc.tensor.transpose(
                        ptp[:], pooled[:, 128 * fo:128 * (fo + 1)], idf[:HT, :HT]
                    )
                    nc.vector.tensor_copy(out=plt[:, fo, :], in_=ptp[:])
                # audio-dim projection
                vtp = ps234.tile([HT, DA], F32, tag="mm", bufs=mmb)
                for fo in range(8):
                    nc.tensor.matmul(
                        vtp[:],
                        plt[:, fo, :],
                        pjt_s[:, fo, :],
                        start=(fo == 0),
                        stop=(fo == 7),
                    )
                vt = s3.tile([HT, DA], F32, tag=f"vt{half}")
                nc.vector.tensor_tensor(
                    out=vt[:], in0=vtp[:], in1=pjb_s[:HT, :],
                    op=mybir.AluOpType.add,
                )
                # s_t = exp(logit_scale) / ||vt||
                vst = s3.tile([HT, 6], F32, tag=f"vs{half}")
                nc.vector.bn_stats(out=vst[:], in_=vt[:])
                vmv = s3.tile([HT, 2], F32, tag=f"vm{half}")
                nc.vector.bn_aggr(out=vmv[:], in_=vst[:])
                vss = s3.tile([HT, 1], F32, tag=f"vq{half}")
                nc.vector.tensor_tensor(
                    out=vss[:], in0=vmv[:, 0:1], in1=vmv[:, 0:1],
                    op=mybir.AluOpType.mult,
                )
                nc.vector.tensor_add(vss[:], vss[:], vmv[:, 1:2])
                nc.scalar.activation(
                    out=vss[:], in_=vss[:],
                    func=mybir.ActivationFunctionType.Sqrt, scale=float(DA),
                )
                st = s3.tile([HT, 1], F32, tag=f"st{half}")
                nc.vector.reciprocal(out=st[:], in_=vss[:])
                nc.vector.tensor_scalar_mul(
                    out=st[:], in0=st[:], scalar1=sca_s[:HT, 0:1]
                )
                # vt^T as bf16 for the similarity matmul
                vttb = s3.tile([128, 4, HT], BF16, tag=f"vb{half}")
                for do in range(4):
                    vtp2 = ps234.tile([128, HT], F32, tag="tr", bufs=trb)
                    nc.tensor.transpose(
                        vtp2[:], vt[:, 128 * do:128 * (do + 1)], idf[:HT, :HT]
                    )
                    nc.vector.tensor_copy(out=vttb[:, do, :], in_=vtp2[:])
                half_state[half] = (vttb, st)

            def emit_sim(half, s3, ps234, mmb=2):
                """Stage 4: similarity vs all audio tokens."""
                t0 = half * HT
                ts_ = slice(t0, t0 + HT)
                vttb, st = half_state[half]
                for lc in range(4):
                    smp = ps234.tile([HT, 512], F32, tag="mm", bufs=mmb)
                    for do in range(4):
                        nc.tensor.matmul(
                            smp[:],
                            vttb[:, do, :],
                            audtb[:, 4 * lc:4 * (lc + 1), do, :],
                            start=(do == 0),
                            stop=(do == 3),
                        )
                    o1 = s3.tile([HT, 512], F32, tag=f"o1{half}", bufs=2)
                    nc.vector.tensor_tensor(
                        out=o1[:], in0=smp[:],
                        in1=rl_b[:HT, 512 * lc:512 * (lc + 1)],
                        op=mybir.AluOpType.mult,
                    )
                    nc.vector.tensor_scalar(
                        out=o1[:], in0=o1[:],
                        scalar1=st[:], scalar2=sca_s[:HT, 1:2],
                        op0=mybir.AluOpType.mult, op1=mybir.AluOpType.add,
                    )
                    nc.scalar.dma_start(out=out[ts_, 512 * lc:512 * (lc + 1)], in_=o1[:])

            # ---- stage 1: per-frame attention pooling in x-space, with the
            # tail stages interleaved to fill DMA-bound PE gaps ----
            with (
                tc.tile_pool(name="xb", bufs=3) as xbp,
                tc.tile_pool(name="xt", bufs=2) as xtp,
                tc.tile_pool(name="sm", bufs=2) as smp_,
                tc.tile_pool(name="at", bufs=2) as atp_,
                tc.tile_pool(name="audp", bufs=1) as audp,
                tc.tile_pool(name="s3", bufs=1) as s3,
                tc.tile_pool(name="ps_xt", bufs=3, space="PSUM") as ps_xt,
                tc.tile_pool(name="ps_sc", bufs=1, space="PSUM") as ps_sc,
                tc.tile_pool(name="ps_cx", bufs=1, space="PSUM") as ps_cx,
                tc.tile_pool(name="ps234", bufs=1, space="PSUM") as ps234,
            ):
                def emit_attn_cx(g, xbg, attn_t):
                    """Attn transpose + cx for group g (emitted one group late
                    so PE isn't head-of-line blocked on g's softmax)."""
                    attn_s = atp_.tile([128, FPG, 2, NH], BF16)
                    for f in range(FPG):
                        for so in range(2):
                            atp = ps_xt.tile([128, NH], BF16, tag="xt")
                            nc.tensor.transpose(
                                atp[:],
                                attn_t[32 * f:32 * f + 16, 128 * so:128 * (so + 1)],
                                idb[32 * f:32 * f + 16, 32 * f:32 * f + 16],
                                tile_position=(32 * f, 0),
                            )
                            nc.vector.tensor_copy(out=attn_s[:, f, so, :], in_=atp[:])
                    # cx^T[c, h] = sum_s xb[s, c] * attn[s, h]
                    for f in range(FPG):
                        cxp = ps_cx.tile([128, 128], F32)
                        for co in range(8):
                            for so in range(2):
                                nc.tensor.matmul(
                                    cxp[:, co * NH:(co + 1) * NH],
                                    xbg[:, f, so, co * 128:(co + 1) * 128],
                                    attn_s[:, f, so, :],
                                    start=(so == 0),
                                    stop=(so == 1),
                                )
                        t = g * FPG + f
                        nc.vector.tensor_copy(
                            out=cxt[:, :, t, :],
                            in_=cxp.rearrange("p (co h) -> p co h", co=8),
                        )

                nxc = 0
                pend = None
                for g in range(n_groups):
                    if g == 2:
                        emit_weight_loads_a()
                    if g == 4:
                        emit_weight_loads_b()
                        emit_audio_load(audp)
                    if g == 5:
                        emit_weight_loads_c()
                    # load FPG frames, cast f32 -> bf16 in DMA; one DMA per
                    # frame so transposes can chase the load frame-by-frame
                    xbg = xbp.tile([128, FPG, 2, DV], BF16)  # [si, f, so, c]
                    for f in range(FPG):
                        base = (g * FPG + f) * S
                        nc.gpsimd.dma_start(
                            out=xbg[:, f, :, :],
                            in_=xv[base:base + S, :].rearrange(
                                "(so si) c -> si so c", so=2
                            ),
                        )
                    # transpose to [ci, f, so, co, si] via TensorE (PE identity
                    # matmul); 8 blocks batched into one PSUM bank, one wide
                    # PSUM->SBUF copy per (f, so), alternating vector/gpsimd.
                    xtg = xtp.tile([128, FPG, 2, 8, 128], BF16)
                    for f in range(FPG):
                        for so in range(2):
                            xt_ps = ps_xt.tile([128, 8, 128], BF16, tag="xt")
                            for co in range(8):
                                nc.tensor.transpose(
                                    xt_ps[:, co, :],
                                    xbg[:, f, so, co * 128:(co + 1) * 128],
                                    idb[:],
                                )
                            if nxc % 2 == 0:
                                nc.vector.tensor_copy(
                                    out=xtg[:, f, so, :, :], in_=xt_ps[:]
                                )
                            else:
                                nc.scalar.activation(
                                    out=xtg[:, f, so, :, :], in_=xt_ps[:],
                                    func=mybir.ActivationFunctionType.Copy,
                                )
                            nxc += 1
                    # scores^T: [16@32f, s=256] per frame, packed on partitions.
                    # ck is zero-padded to M=32 so every partition row of scp is
                    # written (junk-but-finite rows 16..32 of each block).
                    scp = ps_sc.tile([128, S], F32)
                    for f in range(FPG):
                        for co in range(8):
                            nc.tensor.matmul(
                                scp[32 * f:32 * f + 32, :],
                                ck_v[:, co, :],
                                xtg[:, f, :, co, :],
                                start=(co == 0),
                                stop=(co == 7),
                                tile_position=(0, 32 * f),
                            )
                    # softmax over s (free dim), whole group at once (junk rows
                    # stay finite: scores there are 0, exp(0-max)<=1)
                    negm = smp_.tile([128, 1], F32)
                    et = smp_.tile([128, S], F32)
                    esum = smp_.tile([128, 1], F32)
                    rsum = smp_.tile([128, 1], F32)
                    attn_t = atp_.tile([128, S], BF16)  # attn^T [h@32f, s]
                    nc.vector.reduce_max(
                        out=negm[:], in_=scp[:], axis=mybir.AxisListType.X,
                        negate=True,
                    )
                    nc.scalar.activation(
                        out=et[:], in_=scp[:],
                        func=mybir.ActivationFunctionType.Exp,
                        bias=negm[:], scale=1.0, accum_out=esum[:],
                    )
                    nc.vector.reciprocal(out=rsum[:], in_=esum[:])
                    nc.vector.tensor_scalar_mul(attn_t[:], et[:], rsum[:])
                    # previous group's attn transpose + cx (keeps PE fed while
                    # this group's softmax runs on vector/scalar)
                    if pend is not None:
                        emit_attn_cx(*pend)
                    pend = (g, xbg, attn_t)
                    if g == 6:
                        emit_audio_transposes(ps234)
                        emit_stage23(0, s3, ps234)
                        emit_sim(0, s3, ps234)
                emit_attn_cx(*pend)

            # half 1 runs after stage-1 drains; fresh pools with deeper rings
            with (
                tc.tile_pool(name="s3b", bufs=1) as s3b,
                tc.tile_pool(name="ps_t2", bufs=1, space="PSUM") as ps_t2,
            ):
                emit_stage23(1, s3b, ps_t2, mmb=3, trb=3)
                emit_sim(1, s3b, ps_t2, mmb=3)

    nc.compile()
    return nc


def host_fold(probe, wq, wk, bq, wv, bv, wo, bo, ln_g, ln_b, proj_w, proj_b,
              logit_scale, logit_bias):
    """Fold weights on the host into device-friendly layouts."""
    f64 = np.float64
    qvec = probe.reshape(-1).astype(f64) @ wq.astype(f64).T + bq.astype(f64)
    q = qvec.reshape(NH, DH)
    ck = np.stack(
        [q[h] @ wk.astype(f64)[h * DH:(h + 1) * DH, :] for h in range(NH)]
    )  # [16, 1024]
    ck /= np.sqrt(f64(DH))
    # zero-pad heads to M=32 so the scores matmul writes full 32-row blocks
    ckp = np.zeros((32, DV), np.float64)
    ckp[:NH] = ck
    # cktb[ci, co*32+m] = ckp[m, co*128+ci]
    ckt = ckp.T.reshape(8, 128, 32).transpose(1, 0, 2).reshape(128, 256)
    cktb = ckt.astype(ml_dtypes.bfloat16)

    wvt = np.ascontiguousarray(
        wv.T.reshape(8, 128, DV).transpose(1, 0, 2)).astype(ml_dtypes.bfloat16)
    wot = np.ascontiguousarray(
        wo.T.reshape(8, 128, DV).transpose(1, 0, 2)).astype(ml_dtypes.bfloat16)
    # fold LayerNorm gain/bias into the audio projection:
    #   proj(LN_affine(z)) = z @ (proj_w * g)^T + (proj_b + proj_w @ b)
    pw = proj_w.astype(f64) * ln_g.astype(f64)[None, :]
    pb = proj_b.astype(f64) + proj_w.astype(f64) @ ln_b.astype(f64)
    pjt = np.ascontiguousarray(
        pw.T.reshape(8, 128, DA).transpose(1, 0, 2)).astype(ml_dtypes.bfloat16)
    bvp = np.ascontiguousarray(bv.reshape(8, 128).T).astype(np.float32)
    sca = np.array([[np.exp(np.float64(logit_scale[0])), logit_bias[0]]],
                   np.float32)
    return dict(
        cktb=np.ascontiguousarray(cktb),
        wvt=wvt, bvp=bvp, wot=wot,
        bo2=bo.reshape(1, DV).astype(np.float32),
        pjt=pjt, pjb=pb.reshape(1, DA).astype(np.float32),
        sca=sca,
    )


_NC_CACHE = {}


def kernel(video_x, audio_x, probe, wq, wk, wv, bq, bk, bv, wo, bo,
           ln_g, ln_b, proj_w, proj_b, logit_scale, logit_bias, T=None, H=None,
           W=None, **_unused):
    from concourse.bass_utils import run_bass_kernel_spmd

    video_x = np.asarray(video_x, np.float32)
    audio_x = np.asarray(audio_x, np.float32)
    params = host_fold(
        np.asarray(probe, np.float32), np.asarray(wq, np.float32),
        np.asarray(wk, np.float32), np.asarray(bq, np.float32),
        np.asarray(wv, np.float32), np.asarray(bv, np.float32),
        np.asarray(wo, np.float32), np.asarray(bo, np.float32),
        np.asarray(ln_g, np.float32), np.asarray(ln_b, np.float32),
        np.asarray(proj_w, np.float32), np.asarray(proj_b, np.float32),
        np.asarray(logit_scale, np.float32), np.asarray(logit_bias, np.float32),
    )
    if "nc" not in _NC_CACHE:
        _NC_CACHE["nc"] = build_nc()
    nc = _NC_CACHE["nc"]
    in_maps = []
    for b in range(B):
        m = dict(params)
        m["xv"] = np.ascontiguousarray(video_x[b])
        m["aud"] = np.ascontiguousarray(audio_x[b])
        in_maps.append(m)
    res = run_bass_kernel_spmd(nc, in_maps, core_ids=list(range(B)), trace=False)
    return np.stack([res.results[b]["out"] for b in range(B)], axis=0)



# revision 38
# speedup vs baseline: 1.0737x; 1.0166x over previous
"""Trainium2 Bass kernel for ContrastiveVideoAudioSimilarity.

Math (per batch element b, fully folded form):
  q        = probe @ wq.T + bq                      # [1024] -> heads [16, 64]
  ck[h,:]  = q[h] @ wk[h*64:(h+1)*64, :]            # [16, 1024]  (host folded)
  scores   = x @ ck.T / 8                           # [T*S, 16]; bk shift cancels in softmax
  attn     = softmax over S
  cx[t,h]  = sum_s attn[s,h] * x[t,s,:]             # [T, 16, 1024]
  ctx[t,h*64+d] = cx[t,h] @ wv[h*64+d,:] + bv       # per-head V proj of pooled vector
  pooled   = ctx @ wo.T + bo ; LayerNorm ; proj ; L2-normalize both sides; scaled dot.

Sharding: batch B=8, one batch element per NeuronCore (8 cores), params replicated.
Host precomputes folded/transposed weights (CKT, wvT, woT, projT) so the device
never touches wq/wk and needs no on-device weight transposes.
"""

import sys

for _p in ("/opt/trn_rl_repo", "/root/.axon_site/_ro/trn_rl_repo"):
    if _p not in sys.path:
        sys.path.insert(0, _p)

import numpy as np
import ml_dtypes

import concourse.bass as bass
import concourse.tile as tile
from concourse import bacc, mybir
from concourse.masks import make_identity

F32 = mybir.dt.float32
F32R = mybir.dt.float32r
BF16 = mybir.dt.bfloat16

B, T, S, DV, DA, NH, DH, L2 = 8, 32, 256, 1024, 512, 16, 64, 2048
EPS = 1e-6
FPG = 4  # frames per group


def build_nc(n_groups=T // FPG):
    """Build the per-core Bass program. n_groups*FPG = number of frames processed."""
    nT = n_groups * FPG  # frames
    nc = bacc.Bacc("TRN2", target_bir_lowering=False, debug=False)

    xv = nc.dram_tensor("xv", [nT * S, DV], F32, kind="ExternalInput").ap()
    aud = nc.dram_tensor("aud", [L2, DA], F32, kind="ExternalInput").ap()
    cktb = nc.dram_tensor("cktb", [128, 256], BF16, kind="ExternalInput").ap()
    wvt = nc.dram_tensor("wvt", [128, 8, DV], BF16, kind="ExternalInput").ap()
    bvp = nc.dram_tensor("bvp", [128, 8], F32, kind="ExternalInput").ap()
    wot = nc.dram_tensor("wot", [128, 8, DV], BF16, kind="ExternalInput").ap()
    bo2 = nc.dram_tensor("bo2", [1, DV], F32, kind="ExternalInput").ap()
    pjt = nc.dram_tensor("pjt", [128, 8, DA], BF16, kind="ExternalInput").ap()
    pjb = nc.dram_tensor("pjb", [1, DA], F32, kind="ExternalInput").ap()
    sca = nc.dram_tensor("sca", [1, 2], F32, kind="ExternalInput").ap()
    out = nc.dram_tensor("out", [nT, L2], F32, kind="ExternalOutput").ap()

    with tile.TileContext(nc) as tc:
        with (
            tc.tile_pool(name="const", bufs=1) as constp,
            tc.tile_pool(name="persist", bufs=1) as persist,
            tc.tile_pool(name="dram", bufs=1, space="DRAM") as dramp,
        ):
            # ---- constants / params resident in SBUF ----
            idb = constp.tile([128, 128], BF16)
            make_identity(nc, idb)
            idf = constp.tile([128, 128], F32)
            make_identity(nc, idf)
            ck_s = constp.tile([128, 256], BF16)
            nc.gpsimd.dma_start(ck_s[:], cktb)
            ck_v = ck_s.rearrange("p (co m) -> p co m", co=8)
            # weight/bias tiles allocated now, loaded mid-stage-1 (they're not
            # needed before stage 2, and loading them up-front delays group 0)
            wvt_s = persist.tile([128, 8, DV], BF16)
            bvp_s = constp.tile([128, 8], F32)
            wot_s = persist.tile([128, 8, DV], BF16)
            pjt_s = persist.tile([128, 8, DA], BF16)
            bo2_s = constp.tile([nT, DV], F32)
            pjb_s = constp.tile([nT, DA], F32)
            sca_s = constp.tile([nT, 2], F32)

            def bcast_dma(t_, src, parts):
                """Load a [1, free] DRAM row broadcast across `parts` partitions."""
                src_b = bass.AP(
                    tensor=src.tensor, offset=src.offset,
                    ap=[[0, parts]] + list(src.ap[1:]),
                )
                nc.gpsimd.dma_start(out=t_[:], in_=src_b)

            def emit_weight_loads_a():
                nc.gpsimd.dma_start(wvt_s[:], wvt)
                nc.gpsimd.dma_start(bvp_s[:], bvp)

            def emit_weight_loads_b():
                nc.gpsimd.dma_start(wot_s[:], wot)
                bcast_dma(bo2_s, bo2, nT)

            def emit_weight_loads_c():
                nc.gpsimd.dma_start(pjt_s[:], pjt)
                bcast_dma(pjb_s, pjb, nT)
                bcast_dma(sca_s, sca, nT)

            epsb = constp.tile([128, 1], F32)
            nc.vector.memset(epsb, EPS)

            # cx^T accumulator: cxt[ci, co, t, h] = cx[t, h, co*128+ci]
            cxt = persist.tile([128, 8, nT, NH], BF16)

            # audio tiles (filled after stage 1 is issued, so the video DMAs
            # are first in the gpsimd issue queue)
            audtb = persist.tile([128, 16, 4, 128], BF16)  # [d_in, lblk, d_out, l_in]
            rl_b = constp.tile([T, L2], F32)  # 1/||audio_l|| bcast on partitions

            audb = persist.tile([128, 16, DA], BF16)  # [l_in, l_blk, d]

            def emit_audio_load(audp):
                """Audio load + norm stats; interleaved mid-stage-1.

                (audtb transposes happen later on TensorE, after stage 2.)
                """
                nc.gpsimd.dma_start(
                    out=audb[:], in_=aud.rearrange("(b a) d -> a b d", a=128)
                )
                ast = audp.tile([128, 16, 6], F32)
                amv = audp.tile([128, 16, 2], F32)
                for b in range(16):
                    nc.vector.bn_stats(out=ast[:, b, :], in_=audb[:, b, :])
                    nc.vector.bn_aggr(out=amv[:, b, :], in_=ast[:, b, :])
                ssum = audp.tile([128, 16], F32)
                nc.vector.tensor_tensor(
                    out=ssum[:], in0=amv[:, :, 0], in1=amv[:, :, 0],
                    op=mybir.AluOpType.mult,
                )
                nc.vector.tensor_add(ssum[:], ssum[:], amv[:, :, 1])
                # ||a_l|| = sqrt(DA * (var + mu^2))
                nc.scalar.activation(
                    out=ssum[:], in_=ssum[:],
                    func=mybir.ActivationFunctionType.Sqrt, scale=float(DA),
                )
                rml = audp.tile([128, 16], F32)
                nc.vector.reciprocal(out=rml[:], in_=ssum[:])
                # flat l-order roundtrip via DRAM: rl_flat[b*128+a] = rml[a, b]
                rld = dramp.tile([L2], F32)
                nc.scalar.dma_start(
                    out=rld.rearrange("(b a) -> a b", a=128), in_=rml[:]
                )
                nc.gpsimd.dma_start(
                    out=rl_b[:nT],
                    in_=bass.AP(tensor=rld.tensor, offset=rld.offset,
                                ap=[[0, nT]] + list(rld.ap)),
                )

            # ctx^T[e, t] (stage-2 output, filled per half)
            ctxt = persist.tile([128, 8, nT], BF16)
            HT = nT // 2  # frames per tail-half

            def emit_audio_transposes(ps234):
                """audb [l, b, d] -> audtb [d, b, do, l] via TensorE."""
                for b in range(16):
                    au_ps = ps234.tile([128, 4, 128], BF16, tag="tr")
                    for do in range(4):
                        nc.tensor.transpose(
                            au_ps[:, do, :],
                            audb[:, b, do * 128:(do + 1) * 128],
                            idb[:],
                        )
                    if b % 2 == 0:
                        nc.vector.tensor_copy(out=audtb[:, b, :, :], in_=au_ps[:])
                    else:
                        nc.scalar.activation(
                            out=audtb[:, b, :, :], in_=au_ps[:],
                            func=mybir.ActivationFunctionType.Copy,
                        )

            half_state = {}

            def emit_stage23(half, s3, ps234, mmb=2, trb=1):
                """V-proj + wo/LN + audio proj for HT frames."""
                t0 = half * HT
                ts_ = slice(t0, t0 + HT)
                # stage 2: per-head V-projection ctx^T[e, t]
                for eo in range(8):
                    ctp = ps234.tile([128, HT], F32, tag="mm", bufs=mmb)
                    for hh in range(2):
                        h = 2 * eo + hh
                        for co in range(8):
                            nc.tensor.matmul(
                                ctp[64 * hh:64 * (hh + 1), :],
                                wvt_s[:, co, h * DH:(h + 1) * DH],
                                cxt[:, co, ts_, h],
                                start=(co == 0),
                                stop=(co == 7),
                                skip_group_check=True,
                            )
                    nc.vector.tensor_scalar_add(
                        out=ctxt[:, eo, ts_], in0=ctp[:], scalar1=bvp_s[:, eo:eo + 1]
                    )
                # stage 3: wo projection + LayerNorm
                pooled = s3.tile([HT, DV], F32, tag=f"pl{half}")
                for n in range(2):
                    pp = ps234.tile([HT, 512], F32, tag="mm", bufs=mmb)
                    for eo in range(8):
                        nc.tensor.matmul(
                            pp[:],
                            ctxt[:, eo, ts_],
                            wot_s[:, eo, 512 * n:512 * (n + 1)],
                            start=(eo == 0),
                            stop=(eo == 7),
                        )
                    nc.vector.tensor_tensor(
                        out=pooled[:, 512 * n:512 * (n + 1)], in0=pp[:],
                        in1=bo2_s[:HT, 512 * n:512 * (n + 1)],
                        op=mybir.AluOpType.add,
                    )
                lst = s3.tile([HT, 2, 6], F32, tag=f"ls{half}")
                nc.vector.bn_stats(out=lst[:, 0, :], in_=pooled[:, 0:512])
                nc.vector.bn_stats(out=lst[:, 1, :], in_=pooled[:, 512:1024])
                lmv = s3.tile([HT, 2], F32, tag=f"lm{half}")
                nc.vector.bn_aggr(out=lmv[:], in_=lst[:])
                sd = s3.tile([HT, 1], F32, tag=f"sd{half}")
                nc.scalar.activation(
                    out=sd[:], in_=lmv[:, 1:2],
                    func=mybir.ActivationFunctionType.Sqrt, bias=epsb[:HT],
                )
                rstd = s3.tile([HT, 1], F32, tag=f"rs{half}")
                nc.vector.reciprocal(out=rstd[:], in_=sd[:])
                # ln_g/ln_b are folded into pjt/pjb host-side; only z-score here
                nc.vector.tensor_scalar(
                    out=pooled[:], in0=pooled[:],
                    scalar1=lmv[:, 0:1], scalar2=rstd[:],
                    op0=mybir.AluOpType.subtract, op1=mybir.AluOpType.mult,
                )
                # transpose pooled -> [f, t]
                plt = s3.tile([128, 8, HT], BF16, tag=f"pt{half}")
                for fo in range(8):
                    ptp = ps234.tile([128, HT], F32, tag="tr", bufs=trb)
                    nc.tensor.transpose(
                        ptp[:], pooled[:, 128 * fo:128 * (fo + 1)], idf[:HT, :HT]
                    )
                    nc.vector.tensor_copy(out=plt[:, fo, :], in_=ptp[:])
                # audio-dim projection
                vtp = ps234.tile([HT, DA], F32, tag="mm", bufs=mmb)
                for fo in range(8):
                    nc.tensor.matmul(
                        vtp[:],
                        plt[:, fo, :],
                        pjt_s[:, fo, :],
                        start=(fo == 0),
                        stop=(fo == 7),
                    )
                vt = s3.tile([HT, DA], F32, tag=f"vt{half}")
                nc.vector.tensor_tensor(
                    out=vt[:], in0=vtp[:], in1=pjb_s[:HT, :],
                    op=mybir.AluOpType.add,
                )
                # s_t = exp(logit_scale) / ||vt||
                vst = s3.tile([HT, 6], F32, tag=f"vs{half}")
                nc.vector.bn_stats(out=vst[:], in_=vt[:])
                vmv = s3.tile([HT, 2], F32, tag=f"vm{half}")
                nc.vector.bn_aggr(out=vmv[:], in_=vst[:])
                vss = s3.tile([HT, 1], F32, tag=f"vq{half}")
                nc.vector.tensor_tensor(
                    out=vss[:], in0=vmv[:, 0:1], in1=vmv[:, 0:1],
                    op=mybir.AluOpType.mult,
                )
                nc.vector.tensor_add(vss[:], vss[:], vmv[:, 1:2])
                nc.scalar.activation(
                    out=vss[:], in_=vss[:],
                    func=mybir.ActivationFunctionType.Sqrt, scale=float(DA),
                )
                st = s3.tile([HT, 1], F32, tag=f"st{half}")
                nc.vector.reciprocal(out=st[:], in_=vss[:])
                nc.vector.tensor_scalar_mul(
                    out=st[:], in0=st[:], scalar1=sca_s[:HT, 0:1]
                )
                # vt^T as bf16 for the similarity matmul
                vttb = s3.tile([128, 4, HT], BF16, tag=f"vb{half}")
                for do in range(4):
                    vtp2 = ps234.tile([128, HT], F32, tag="tr", bufs=trb)
                    nc.tensor.transpose(
                        vtp2[:], vt[:, 128 * do:128 * (do + 1)], idf[:HT, :HT]
                    )
                    nc.vector.tensor_copy(out=vttb[:, do, :], in_=vtp2[:])
                half_state[half] = (vttb, st)

            def emit_sim(half, s3, ps234, mmb=2):
                """Stage 4: similarity vs all audio tokens."""
                t0 = half * HT
                ts_ = slice(t0, t0 + HT)
                vttb, st = half_state[half]
                for lc in range(4):
                    smp = ps234.tile([HT, 512], F32, tag="mm", bufs=mmb)
                    for do in range(4):
                        nc.tensor.matmul(
                            smp[:],
                            vttb[:, do, :],
                            audtb[:, 4 * lc:4 * (lc + 1), do, :],
                            start=(do == 0),
                            stop=(do == 3),
                        )
                    o1 = s3.tile([HT, 512], F32, tag=f"o1{half}", bufs=2)
                    nc.vector.tensor_tensor(
                        out=o1[:], in0=smp[:],
                        in1=rl_b[:HT, 512 * lc:512 * (lc + 1)],
                        op=mybir.AluOpType.mult,
                    )
                    nc.vector.tensor_scalar(
                        out=o1[:], in0=o1[:],
                        scalar1=st[:], scalar2=sca_s[:HT, 1:2],
                        op0=mybir.AluOpType.mult, op1=mybir.AluOpType.add,
                    )
                    nc.scalar.dma_start(out=out[ts_, 512 * lc:512 * (lc + 1)], in_=o1[:])

            # ---- stage 1: per-frame attention pooling in x-space, with the
            # tail stages interleaved to fill DMA-bound PE gaps ----
            with (
                tc.tile_pool(name="xb", bufs=3) as xbp,
                tc.tile_pool(name="xt", bufs=2) as xtp,
                tc.tile_pool(name="sm", bufs=2) as smp_,
                tc.tile_pool(name="at", bufs=2) as atp_,
                tc.tile_pool(name="audp", bufs=1) as audp,
                tc.tile_pool(name="s3", bufs=1) as s3,
                tc.tile_pool(name="ps_xt", bufs=3, space="PSUM") as ps_xt,
                tc.tile_pool(name="ps_sc", bufs=1, space="PSUM") as ps_sc,
                tc.tile_pool(name="ps_cx", bufs=1, space="PSUM") as ps_cx,
                tc.tile_pool(name="ps234", bufs=1, space="PSUM") as ps234,
            ):
                def emit_attn_cx(g, xbg, attn_t):
                    """Attn transpose + cx for group g (emitted one group late
                    so PE isn't head-of-line blocked on g's softmax)."""
                    attn_s = atp_.tile([128, FPG, 2, NH], BF16)
                    for f in range(FPG):
                        for so in range(2):
                            atp = ps_xt.tile([128, NH], BF16, tag="xt")
                            nc.tensor.transpose(
                                atp[:],
                                attn_t[32 * f:32 * f + 16, 128 * so:128 * (so + 1)],
                                idb[32 * f:32 * f + 16, 32 * f:32 * f + 16],
                                tile_position=(32 * f, 0),
                            )
                            nc.vector.tensor_copy(out=attn_s[:, f, so, :], in_=atp[:])
                    # cx^T[c, h] = sum_s xb[s, c] * attn[s, h]
                    for f in range(FPG):
                        cxp = ps_cx.tile([128, 128], F32)
                        for co in range(8):
                            for so in range(2):
                                nc.tensor.matmul(
                                    cxp[:, co * NH:(co + 1) * NH],
                                    xbg[:, f, so, co * 128:(co + 1) * 128],
                                    attn_s[:, f, so, :],
                                    start=(so == 0),
                                    stop=(so == 1),
                                )
                        t = g * FPG + f
                        nc.vector.tensor_copy(
                            out=cxt[:, :, t, :],
                            in_=cxp.rearrange("p (co h) -> p co h", co=8),
                        )

                nxc = 0
                pend = None
                for g in range(n_groups):
                    if g == 4:
                        emit_weight_loads_a()
                    if g == 5:
                        emit_weight_loads_b()
                        emit_audio_load(audp)
                    if g == 6:
                        emit_weight_loads_c()
                    # load FPG frames, cast f32 -> bf16 in DMA; one DMA per
                    # frame so transposes can chase the load frame-by-frame
                    xbg = xbp.tile([128, FPG, 2, DV], BF16)  # [si, f, so, c]
                    for f in range(FPG):
                        base = (g * FPG + f) * S
                        nc.gpsimd.dma_start(
                            out=xbg[:, f, :, :],
                            in_=xv[base:base + S, :].rearrange(
                                "(so si) c -> si so c", so=2
                            ),
                        )
                    # transpose to [ci, f, so, co, si] via TensorE (PE identity
                    # matmul); 8 blocks batched into one PSUM bank, one wide
                    # PSUM->SBUF copy per (f, so), alternating vector/gpsimd.
                    xtg = xtp.tile([128, FPG, 2, 8, 128], BF16)
                    for f in range(FPG):
                        for so in range(2):
                            xt_ps = ps_xt.tile([128, 8, 128], BF16, tag="xt")
                            for co in range(8):
                                nc.tensor.transpose(
                                    xt_ps[:, co, :],
                                    xbg[:, f, so, co * 128:(co + 1) * 128],
                                    idb[:],
                                )
                            if nxc % 2 == 0:
                                nc.vector.tensor_copy(
                                    out=xtg[:, f, so, :, :], in_=xt_ps[:]
                                )
                            else:
                                nc.scalar.activation(
                                    out=xtg[:, f, so, :, :], in_=xt_ps[:],
                                    func=mybir.ActivationFunctionType.Copy,
                                )
                            nxc += 1
                    # scores^T: [16@32f, s=256] per frame, packed on partitions.
                    # ck is zero-padded to M=32 so every partition row of scp is
                    # written (junk-but-finite rows 16..32 of each block).
                    scp = ps_sc.tile([128, S], F32)
                    for f in range(FPG):
                        for co in range(8):
                            nc.tensor.matmul(
                                scp[32 * f:32 * f + 32, :],
                                ck_v[:, co, :],
                                xtg[:, f, :, co, :],
                                start=(co == 0),
                                stop=(co == 7),
                                tile_position=(0, 32 * f),
                            )
                    # softmax over s (free dim), whole group at once (junk rows
                    # stay finite: scores there are 0, exp(0-max)<=1)
                    negm = smp_.tile([128, 1], F32)
                    et = smp_.tile([128, S], F32)
                    esum = smp_.tile([128, 1], F32)
                    rsum = smp_.tile([128, 1], F32)
                    attn_t = atp_.tile([128, S], BF16)  # attn^T [h@32f, s]
                    nc.vector.reduce_max(
                        out=negm[:], in_=scp[:], axis=mybir.AxisListType.X,
                        negate=True,
                    )
                    nc.scalar.activation(
                        out=et[:], in_=scp[:],
                        func=mybir.ActivationFunctionType.Exp,
                        bias=negm[:], scale=1.0, accum_out=esum[:],
                    )
                    nc.vector.reciprocal(out=rsum[:], in_=esum[:])
                    nc.vector.tensor_scalar_mul(attn_t[:], et[:], rsum[:])
                    # previous group's attn transpose + cx (keeps PE fed while
                    # this group's softmax runs on vector/scalar)
                    if pend is not None:
                        emit_attn_cx(*pend)
                    pend = (g, xbg, attn_t)
                    if g == 6:
                        emit_audio_transposes(ps234)
                        emit_stage23(0, s3, ps234)
                        emit_sim(0, s3, ps234)
                emit_attn_cx(*pend)

            # half 1 runs after stage-1 drains; fresh pools with deeper rings
            with (
                tc.tile_pool(name="s3b", bufs=1) as s3b,
                tc.tile_pool(name="ps_t2", bufs=1, space="PSUM") as ps_t2,
            ):
                emit_stage23(1, s3b, ps_t2, mmb=3, trb=3)
                emit_sim(1, s3b, ps_t2, mmb=3)

    nc.compile()
    return nc


def host_fold(probe, wq, wk, bq, wv, bv, wo, bo, ln_g, ln_b, proj_w, proj_b,
              logit_scale, logit_bias):
    """Fold weights on the host into device-friendly layouts."""
    f64 = np.float64
    qvec = probe.reshape(-1).astype(f64) @ wq.astype(f64).T + bq.astype(f64)
    q = qvec.reshape(NH, DH)
    ck = np.stack(
        [q[h] @ wk.astype(f64)[h * DH:(h + 1) * DH, :] for h in range(NH)]
    )  # [16, 1024]
    ck /= np.sqrt(f64(DH))
    # zero-pad heads to M=32 so the scores matmul writes full 32-row blocks
    ckp = np.zeros((32, DV), np.float64)
    ckp[:NH] = ck
    # cktb[ci, co*32+m] = ckp[m, co*128+ci]
    ckt = ckp.T.reshape(8, 128, 32).transpose(1, 0, 2).reshape(128, 256)
    cktb = ckt.astype(ml_dtypes.bfloat16)

    wvt = np.ascontiguousarray(
        wv.T.reshape(8, 128, DV).transpose(1, 0, 2)).astype(ml_dtypes.bfloat16)
    wot = np.ascontiguousarray(
        wo.T.reshape(8, 128, DV).transpose(1, 0, 2)).astype(ml_dtypes.bfloat16)
    # fold LayerNorm gain/bias into the audio projection:
    #   proj(LN_affine(z)) = z @ (proj_w * g)^T + (proj_b + proj_w @ b)
    pw = proj_w.astype(f64) * ln_g.astype(f64)[None, :]
    pb = proj_b.astype(f64) + proj_w.astype(f64) @ ln_b.astype(f64)
    pjt = np.ascontiguousarray(
        pw.T.reshape(8, 128, DA).transpose(1, 0, 2)).astype(ml_dtypes.bfloat16)
    bvp = np.ascontiguousarray(bv.reshape(8, 128).T).astype(np.float32)
    sca = np.array([[np.exp(np.float64(logit_scale[0])), logit_bias[0]]],
                   np.float32)
    return dict(
        cktb=np.ascontiguousarray(cktb),
        wvt=wvt, bvp=bvp, wot=wot,
        bo2=bo.reshape(1, DV).astype(np.float32),
        pjt=pjt, pjb=pb.reshape(1, DA).astype(np.float32),
        sca=sca,
    )


_NC_CACHE = {}


def kernel(video_x, audio_x, probe, wq, wk, wv, bq, bk, bv, wo, bo,
           ln_g, ln_b, proj_w, proj_b, logit_scale, logit_bias, T=None, H=None,
           W=None, **_unused):
    from concourse.bass_utils import run_bass_kernel_spmd

    video_x = np.asarray(video_x, np.float32)
    audio_x = np.asarray(audio_x, np.float32)
    params = host_fold(
        np.asarray(probe, np.float32), np.asarray(wq, np.float32),
        np.asarray(wk, np.float32), np.asarray(bq, np.float32),
        np.asarray(wv, np.float32), np.asarray(bv, np.float32),
        np.asarray(wo, np.float32), np.asarray(bo, np.float32),
        np.asarray(ln_g, np.float32), np.asarray(ln_b, np.float32),
        np.asarray(proj_w, np.float32), np.asarray(proj_b, np.float32),
        np.asarray(logit_scale, np.float32), np.asarray(logit_bias, np.float32),
    )
    if "nc" not in _NC_CACHE:
        _NC_CACHE["nc"] = build_nc()
    nc = _NC_CACHE["nc"]
    in_maps = []
    for b in range(B):
        m = dict(params)
        m["xv"] = np.ascontiguousarray(video_x[b])
        m["aud"] = np.ascontiguousarray(audio_x[b])
        in_maps.append(m)
    res = run_bass_kernel_spmd(nc, in_maps, core_ids=list(range(B)), trace=False)
    return np.stack([res.results[b]["out"] for b in range(B)], axis=0)



# revision 39
# speedup vs baseline: 1.1720x; 1.0916x over previous
"""Trainium2 Bass kernel for ContrastiveVideoAudioSimilarity.

Math (per batch element b, fully folded form):
  q        = probe @ wq.T + bq                      # [1024] -> heads [16, 64]
  ck[h,:]  = q[h] @ wk[h*64:(h+1)*64, :]            # [16, 1024]  (host folded)
  scores   = x @ ck.T / 8                           # [T*S, 16]; bk shift cancels in softmax
  attn     = softmax over S
  cx[t,h]  = sum_s attn[s,h] * x[t,s,:]             # [T, 16, 1024]
  ctx[t,h*64+d] = cx[t,h] @ wv[h*64+d,:] + bv       # per-head V proj of pooled vector
  pooled   = ctx @ wo.T + bo ; LayerNorm ; proj ; L2-normalize both sides; scaled dot.

Sharding: batch B=8, one batch element per NeuronCore (8 cores), params replicated.
Host precomputes folded/transposed weights (CKT, wvT, woT, projT) so the device
never touches wq/wk and needs no on-device weight transposes.
"""

import sys

for _p in ("/opt/trn_rl_repo", "/root/.axon_site/_ro/trn_rl_repo"):
    if _p not in sys.path:
        sys.path.insert(0, _p)

import numpy as np
import ml_dtypes

import concourse.bass as bass
import concourse.tile as tile
from concourse import bacc, mybir
from concourse.masks import make_identity

F32 = mybir.dt.float32
F32R = mybir.dt.float32r
BF16 = mybir.dt.bfloat16

B, T, S, DV, DA, NH, DH, L2 = 8, 32, 256, 1024, 512, 16, 64, 2048
EPS = 1e-6
FPG = 4  # frames per group


def build_nc(n_groups=T // FPG):
    """Build the per-core Bass program. n_groups*FPG = number of frames processed."""
    nT = n_groups * FPG  # frames
    nc = bacc.Bacc("TRN2", target_bir_lowering=False, debug=False)

    xv = nc.dram_tensor("xv", [nT * S, DV], F32, kind="ExternalInput").ap()
    aud = nc.dram_tensor("aud", [L2, DA], F32, kind="ExternalInput").ap()
    cktb = nc.dram_tensor("cktb", [128, 256], BF16, kind="ExternalInput").ap()
    wvt = nc.dram_tensor("wvt", [128, 8, DV], BF16, kind="ExternalInput").ap()
    wot = nc.dram_tensor("wot", [128, 8, DV], BF16, kind="ExternalInput").ap()
    bo2 = nc.dram_tensor("bo2", [1, DV], F32, kind="ExternalInput").ap()
    pjt = nc.dram_tensor("pjt", [128, 8, DA], BF16, kind="ExternalInput").ap()
    pjb = nc.dram_tensor("pjb", [1, DA], F32, kind="ExternalInput").ap()
    sca = nc.dram_tensor("sca", [1, 2], F32, kind="ExternalInput").ap()
    out = nc.dram_tensor("out", [nT, L2], F32, kind="ExternalOutput").ap()

    with tile.TileContext(nc) as tc:
        with (
            tc.tile_pool(name="const", bufs=1) as constp,
            tc.tile_pool(name="persist", bufs=1) as persist,
            tc.tile_pool(name="dram", bufs=1, space="DRAM") as dramp,
        ):
            # ---- constants / params resident in SBUF ----
            idb = constp.tile([128, 128], BF16)
            make_identity(nc, idb)
            idf = constp.tile([128, 128], F32)
            make_identity(nc, idf)
            ck_s = constp.tile([128, 256], BF16)
            nc.gpsimd.dma_start(ck_s[:], cktb)
            ck_v = ck_s.rearrange("p (co m) -> p co m", co=8)
            # weight/bias tiles allocated now, loaded mid-stage-1 (they're not
            # needed before stage 2, and loading them up-front delays group 0)
            wvt_s = persist.tile([128, 8, DV], BF16)
            wot_s = persist.tile([128, 8, DV], BF16)
            pjt_s = persist.tile([128, 8, DA], BF16)
            bo2_s = constp.tile([nT, DV], F32)
            pjb_s = constp.tile([nT, DA], F32)
            sca_s = constp.tile([nT, 2], F32)

            def bcast_dma(t_, src, parts):
                """Load a [1, free] DRAM row broadcast across `parts` partitions."""
                src_b = bass.AP(
                    tensor=src.tensor, offset=src.offset,
                    ap=[[0, parts]] + list(src.ap[1:]),
                )
                nc.gpsimd.dma_start(out=t_[:], in_=src_b)

            def emit_weight_loads_a():
                nc.gpsimd.dma_start(wvt_s[:], wvt)

            def emit_weight_loads_b():
                nc.gpsimd.dma_start(wot_s[:], wot)
                bcast_dma(bo2_s, bo2, nT)

            def emit_weight_loads_c():
                nc.gpsimd.dma_start(pjt_s[:], pjt)
                bcast_dma(pjb_s, pjb, nT)
                bcast_dma(sca_s, sca, nT)

            epsb = constp.tile([128, 1], F32)
            nc.vector.memset(epsb, EPS)

            # cx^T accumulator: cxt[ci, co, t, h] = cx[t, h, co*128+ci]
            cxt = persist.tile([128, 8, nT, NH], BF16)

            # audio tiles (filled after stage 1 is issued, so the video DMAs
            # are first in the gpsimd issue queue)
            audtb = persist.tile([128, 16, 4, 128], BF16)  # [d_in, lblk, d_out, l_in]

            audb = persist.tile([128, 16, DA], BF16)  # [l_in, l_blk, d]

            def emit_audio_load(audp):
                """Audio load + norm stats; interleaved mid-stage-1.

                (audtb transposes happen later on TensorE, after stage 2.)
                """
                nc.gpsimd.dma_start(
                    out=audb[:], in_=aud.rearrange("(b a) d -> a b d", a=128)
                )
                ast = audp.tile([128, 16, 6], F32)
                amv = audp.tile([128, 16, 2], F32)
                for b in range(16):
                    nc.vector.bn_stats(out=ast[:, b, :], in_=audb[:, b, :])
                    nc.vector.bn_aggr(out=amv[:, b, :], in_=ast[:, b, :])
                ssum = audp.tile([128, 16], F32)
                nc.vector.tensor_tensor(
                    out=ssum[:], in0=amv[:, :, 0], in1=amv[:, :, 0],
                    op=mybir.AluOpType.mult,
                )
                nc.vector.tensor_add(ssum[:], ssum[:], amv[:, :, 1])
                # ||a_l|| = sqrt(DA * (var + mu^2))
                nc.scalar.activation(
                    out=ssum[:], in_=ssum[:],
                    func=mybir.ActivationFunctionType.Sqrt, scale=float(DA),
                )
                rml = audp.tile([128, 16], F32)
                nc.vector.reciprocal(out=rml[:], in_=ssum[:])
                # scale audio rows by 1/||a_l|| in place (folds the o1
                # per-token normalization into audtb)
                for b in range(16):
                    if b % 2 == 0:
                        nc.vector.tensor_scalar_mul(
                            out=audb[:, b, :], in0=audb[:, b, :],
                            scalar1=rml[:, b:b + 1],
                        )
                    else:
                        nc.scalar.activation(
                            out=audb[:, b, :], in_=audb[:, b, :],
                            func=mybir.ActivationFunctionType.Copy,
                            scale=rml[:, b:b + 1],
                        )

            # ctx^T[e, t] (stage-2 output, filled per half)
            ctxt = persist.tile([128, 8, nT], BF16)
            HT = nT // 2  # frames per tail-half

            def emit_audio_transposes(ps234):
                """audb [l, b, d] -> audtb [d, b, do, l] via TensorE."""
                for b in range(16):
                    au_ps = ps234.tile([128, 4, 128], BF16, tag="tr")
                    for do in range(4):
                        nc.tensor.transpose(
                            au_ps[:, do, :],
                            audb[:, b, do * 128:(do + 1) * 128],
                            idb[:],
                        )
                    if b % 2 == 0:
                        nc.vector.tensor_copy(out=audtb[:, b, :, :], in_=au_ps[:])
                    else:
                        nc.scalar.activation(
                            out=audtb[:, b, :, :], in_=au_ps[:],
                            func=mybir.ActivationFunctionType.Copy,
                        )

            half_state = {}

            def emit_stage23(half, s3, ps234, mmb=2, trb=1):
                """V-proj + wo/LN + audio proj for HT frames."""
                t0 = half * HT
                ts_ = slice(t0, t0 + HT)
                # stage 2: per-head V-projection ctx^T[e, t]
                for eo in range(8):
                    ctp = ps234.tile([128, HT], F32, tag="mm", bufs=mmb)
                    for hh in range(2):
                        h = 2 * eo + hh
                        for co in range(8):
                            nc.tensor.matmul(
                                ctp[64 * hh:64 * (hh + 1), :],
                                wvt_s[:, co, h * DH:(h + 1) * DH],
                                cxt[:, co, ts_, h],
                                start=(co == 0),
                                stop=(co == 7),
                                skip_group_check=True,
                            )
                    if eo % 2 == 0:
                        nc.vector.tensor_copy(out=ctxt[:, eo, ts_], in_=ctp[:])
                    else:
                        nc.scalar.activation(
                            out=ctxt[:, eo, ts_], in_=ctp[:],
                            func=mybir.ActivationFunctionType.Copy,
                        )
                # stage 3: wo projection + LayerNorm
                pooled = s3.tile([HT, DV], F32, tag=f"pl{half}")
                for n in range(2):
                    pp = ps234.tile([HT, 512], F32, tag="mm", bufs=mmb)
                    for eo in range(8):
                        nc.tensor.matmul(
                            pp[:],
                            ctxt[:, eo, ts_],
                            wot_s[:, eo, 512 * n:512 * (n + 1)],
                            start=(eo == 0),
                            stop=(eo == 7),
                        )
                    nc.vector.tensor_tensor(
                        out=pooled[:, 512 * n:512 * (n + 1)], in0=pp[:],
                        in1=bo2_s[:HT, 512 * n:512 * (n + 1)],
                        op=mybir.AluOpType.add,
                    )
                lst = s3.tile([HT, 2, 6], F32, tag=f"ls{half}")
                nc.vector.bn_stats(out=lst[:, 0, :], in_=pooled[:, 0:512])
                nc.vector.bn_stats(out=lst[:, 1, :], in_=pooled[:, 512:1024])
                lmv = s3.tile([HT, 2], F32, tag=f"lm{half}")
                nc.vector.bn_aggr(out=lmv[:], in_=lst[:])
                sd = s3.tile([HT, 1], F32, tag=f"sd{half}")
                nc.scalar.activation(
                    out=sd[:], in_=lmv[:, 1:2],
                    func=mybir.ActivationFunctionType.Sqrt, bias=epsb[:HT],
                )
                rstd = s3.tile([HT, 1], F32, tag=f"rs{half}")
                nc.vector.reciprocal(out=rstd[:], in_=sd[:])
                # ln_g/ln_b are folded into pjt/pjb host-side; only z-score here
                nc.vector.tensor_scalar(
                    out=pooled[:], in0=pooled[:],
                    scalar1=lmv[:, 0:1], scalar2=rstd[:],
                    op0=mybir.AluOpType.subtract, op1=mybir.AluOpType.mult,
                )
                # transpose pooled -> [f, t]
                plt = s3.tile([128, 8, HT], BF16, tag=f"pt{half}")
                for fo in range(8):
                    ptp = ps234.tile([128, HT], F32, tag="tr", bufs=trb)
                    nc.tensor.transpose(
                        ptp[:], pooled[:, 128 * fo:128 * (fo + 1)], idf[:HT, :HT]
                    )
                    nc.vector.tensor_copy(out=plt[:, fo, :], in_=ptp[:])
                # audio-dim projection
                vtp = ps234.tile([HT, DA], F32, tag="mm", bufs=mmb)
                for fo in range(8):
                    nc.tensor.matmul(
                        vtp[:],
                        plt[:, fo, :],
                        pjt_s[:, fo, :],
                        start=(fo == 0),
                        stop=(fo == 7),
                    )
                vt = s3.tile([HT, DA], F32, tag=f"vt{half}")
                nc.vector.tensor_tensor(
                    out=vt[:], in0=vtp[:], in1=pjb_s[:HT, :],
                    op=mybir.AluOpType.add,
                )
                # s_t = exp(logit_scale) / ||vt||
                vst = s3.tile([HT, 6], F32, tag=f"vs{half}")
                nc.vector.bn_stats(out=vst[:], in_=vt[:])
                vmv = s3.tile([HT, 2], F32, tag=f"vm{half}")
                nc.vector.bn_aggr(out=vmv[:], in_=vst[:])
                vss = s3.tile([HT, 1], F32, tag=f"vq{half}")
                nc.vector.tensor_tensor(
                    out=vss[:], in0=vmv[:, 0:1], in1=vmv[:, 0:1],
                    op=mybir.AluOpType.mult,
                )
                nc.vector.tensor_add(vss[:], vss[:], vmv[:, 1:2])
                nc.scalar.activation(
                    out=vss[:], in_=vss[:],
                    func=mybir.ActivationFunctionType.Sqrt, scale=float(DA),
                )
                st = s3.tile([HT, 1], F32, tag=f"st{half}")
                nc.vector.reciprocal(out=st[:], in_=vss[:])
                nc.vector.tensor_scalar_mul(
                    out=st[:], in0=st[:], scalar1=sca_s[:HT, 0:1]
                )
                # vt^T as bf16 for the similarity matmul
                vttb = s3.tile([128, 4, HT], BF16, tag=f"vb{half}")
                for do in range(4):
                    vtp2 = ps234.tile([128, HT], F32, tag="tr", bufs=trb)
                    nc.tensor.transpose(
                        vtp2[:], vt[:, 128 * do:128 * (do + 1)], idf[:HT, :HT]
                    )
                    nc.vector.tensor_copy(out=vttb[:, do, :], in_=vtp2[:])
                half_state[half] = (vttb, st)

            def emit_sim(half, s3, ps234, mmb=2):
                """Stage 4: similarity vs all audio tokens."""
                t0 = half * HT
                ts_ = slice(t0, t0 + HT)
                vttb, st = half_state[half]
                for lc in range(4):
                    smp = ps234.tile([HT, 512], F32, tag="mm", bufs=mmb)
                    for do in range(4):
                        nc.tensor.matmul(
                            smp[:],
                            vttb[:, do, :],
                            audtb[:, 4 * lc:4 * (lc + 1), do, :],
                            start=(do == 0),
                            stop=(do == 3),
                        )
                    o1 = s3.tile([HT, 512], F32, tag=f"o1{half}", bufs=2)
                    nc.vector.tensor_scalar(
                        out=o1[:], in0=smp[:],
                        scalar1=st[:], scalar2=sca_s[:HT, 1:2],
                        op0=mybir.AluOpType.mult, op1=mybir.AluOpType.add,
                    )
                    nc.scalar.dma_start(out=out[ts_, 512 * lc:512 * (lc + 1)], in_=o1[:])

            # ---- stage 1: per-frame attention pooling in x-space, with the
            # tail stages interleaved to fill DMA-bound PE gaps ----
            with (
                tc.tile_pool(name="xb", bufs=3) as xbp,
                tc.tile_pool(name="xt", bufs=2) as xtp,
                tc.tile_pool(name="sm", bufs=2) as smp_,
                tc.tile_pool(name="at", bufs=2) as atp_,
                tc.tile_pool(name="audp", bufs=1) as audp,
                tc.tile_pool(name="s3", bufs=1) as s3,
                tc.tile_pool(name="ps_xt", bufs=3, space="PSUM") as ps_xt,
                tc.tile_pool(name="ps_sc", bufs=1, space="PSUM") as ps_sc,
                tc.tile_pool(name="ps_cx", bufs=1, space="PSUM") as ps_cx,
                tc.tile_pool(name="ps234", bufs=1, space="PSUM") as ps234,
            ):
                def emit_attn_cx(g, xbg, attn_t):
                    """Attn transpose + cx for group g (emitted one group late
                    so PE isn't head-of-line blocked on g's softmax)."""
                    attn_s = atp_.tile([128, FPG, 2, NH], BF16)
                    for f in range(FPG):
                        for so in range(2):
                            atp = ps_xt.tile([128, NH], BF16, tag="xt")
                            nc.tensor.transpose(
                                atp[:],
                                attn_t[32 * f:32 * f + 16, 128 * so:128 * (so + 1)],
                                idb[32 * f:32 * f + 16, 32 * f:32 * f + 16],
                                tile_position=(32 * f, 0),
                            )
                            nc.vector.tensor_copy(out=attn_s[:, f, so, :], in_=atp[:])
                    # cx^T[c, h] = sum_s xb[s, c] * attn[s, h]
                    for f in range(FPG):
                        cxp = ps_cx.tile([128, 128], F32)
                        for co in range(8):
                            for so in range(2):
                                nc.tensor.matmul(
                                    cxp[:, co * NH:(co + 1) * NH],
                                    xbg[:, f, so, co * 128:(co + 1) * 128],
                                    attn_s[:, f, so, :],
                                    start=(so == 0),
                                    stop=(so == 1),
                                )
                        t = g * FPG + f
                        nc.vector.tensor_copy(
                            out=cxt[:, :, t, :],
                            in_=cxp.rearrange("p (co h) -> p co h", co=8),
                        )

                nxc = 0
                pend = None
                for g in range(n_groups):
                    if g == 4:
                        emit_weight_loads_a()
                    if g == 5:
                        emit_weight_loads_b()
                        emit_audio_load(audp)
                    if g == 6:
                        emit_weight_loads_c()
                    # load FPG frames, cast f32 -> bf16 in DMA; one DMA per
                    # frame so transposes can chase the load frame-by-frame
                    xbg = xbp.tile([128, FPG, 2, DV], BF16)  # [si, f, so, c]
                    for f in range(FPG):
                        base = (g * FPG + f) * S
                        nc.gpsimd.dma_start(
                            out=xbg[:, f, :, :],
                            in_=xv[base:base + S, :].rearrange(
                                "(so si) c -> si so c", so=2
                            ),
                        )
                    # transpose to [ci, f, so, co, si] via TensorE (PE identity
                    # matmul); 8 blocks batched into one PSUM bank, one wide
                    # PSUM->SBUF copy per (f, so), alternating vector/gpsimd.
                    xtg = xtp.tile([128, FPG, 2, 8, 128], BF16)
                    for f in range(FPG):
                        for so in range(2):
                            xt_ps = ps_xt.tile([128, 8, 128], BF16, tag="xt")
                            for co in range(8):
                                nc.tensor.transpose(
                                    xt_ps[:, co, :],
                                    xbg[:, f, so, co * 128:(co + 1) * 128],
                                    idb[:],
                                )
                            if nxc % 2 == 0:
                                nc.vector.tensor_copy(
                                    out=xtg[:, f, so, :, :], in_=xt_ps[:]
                                )
                            else:
                                nc.scalar.activation(
                                    out=xtg[:, f, so, :, :], in_=xt_ps[:],
                                    func=mybir.ActivationFunctionType.Copy,
                                )
                            nxc += 1
                    # scores^T: [16@32f, s=256] per frame, packed on partitions.
                    # ck is zero-padded to M=32 so every partition row of scp is
                    # written (junk-but-finite rows 16..32 of each block).
                    scp = ps_sc.tile([128, S], F32)
                    for f in range(FPG):
                        for co in range(8):
                            nc.tensor.matmul(
                                scp[32 * f:32 * f + 32, :],
                                ck_v[:, co, :],
                                xtg[:, f, :, co, :],
                                start=(co == 0),
                                stop=(co == 7),
                                tile_position=(0, 32 * f),
                            )
                    # softmax over s (free dim), whole group at once (junk rows
                    # stay finite: scores there are 0, exp(0-max)<=1)
                    negm = smp_.tile([128, 1], F32)
                    et = smp_.tile([128, S], F32)
                    esum = smp_.tile([128, 1], F32)
                    rsum = smp_.tile([128, 1], F32)
                    attn_t = atp_.tile([128, S], BF16)  # attn^T [h@32f, s]
                    nc.vector.reduce_max(
                        out=negm[:], in_=scp[:], axis=mybir.AxisListType.X,
                        negate=True,
                    )
                    nc.scalar.activation(
                        out=et[:], in_=scp[:],
                        func=mybir.ActivationFunctionType.Exp,
                        bias=negm[:], scale=1.0, accum_out=esum[:],
                    )
                    nc.vector.reciprocal(out=rsum[:], in_=esum[:])
                    nc.vector.tensor_scalar_mul(attn_t[:], et[:], rsum[:])
                    # previous group's attn transpose + cx (keeps PE fed while
                    # this group's softmax runs on vector/scalar)
                    if pend is not None:
                        emit_attn_cx(*pend)
                    pend = (g, xbg, attn_t)
                    if g == 6:
                        emit_audio_transposes(ps234)
                        emit_stage23(0, s3, ps234)
                        emit_sim(0, s3, ps234)
                emit_attn_cx(*pend)

            # half 1 runs after stage-1 drains; fresh pools with deeper rings
            with (
                tc.tile_pool(name="s3b", bufs=1) as s3b,
                tc.tile_pool(name="ps_t2", bufs=1, space="PSUM") as ps_t2,
            ):
                emit_stage23(1, s3b, ps_t2, mmb=3, trb=3)
                emit_sim(1, s3b, ps_t2, mmb=3)

    nc.compile()
    return nc


def host_fold(probe, wq, wk, bq, wv, bv, wo, bo, ln_g, ln_b, proj_w, proj_b,
              logit_scale, logit_bias):
    """Fold weights on the host into device-friendly layouts."""
    f64 = np.float64
    qvec = probe.reshape(-1).astype(f64) @ wq.astype(f64).T + bq.astype(f64)
    q = qvec.reshape(NH, DH)
    ck = np.stack(
        [q[h] @ wk.astype(f64)[h * DH:(h + 1) * DH, :] for h in range(NH)]
    )  # [16, 1024]
    ck /= np.sqrt(f64(DH))
    # zero-pad heads to M=32 so the scores matmul writes full 32-row blocks
    ckp = np.zeros((32, DV), np.float64)
    ckp[:NH] = ck
    # cktb[ci, co*32+m] = ckp[m, co*128+ci]
    ckt = ckp.T.reshape(8, 128, 32).transpose(1, 0, 2).reshape(128, 256)
    cktb = ckt.astype(ml_dtypes.bfloat16)

    wvt = np.ascontiguousarray(
        wv.T.reshape(8, 128, DV).transpose(1, 0, 2)).astype(ml_dtypes.bfloat16)
    wot = np.ascontiguousarray(
        wo.T.reshape(8, 128, DV).transpose(1, 0, 2)).astype(ml_dtypes.bfloat16)
    # fold LayerNorm gain/bias into the audio projection:
    #   proj(LN_affine(z)) = z @ (proj_w * g)^T + (proj_b + proj_w @ b)
    pw = proj_w.astype(f64) * ln_g.astype(f64)[None, :]
    pb = proj_b.astype(f64) + proj_w.astype(f64) @ ln_b.astype(f64)
    pjt = np.ascontiguousarray(
        pw.T.reshape(8, 128, DA).transpose(1, 0, 2)).astype(ml_dtypes.bfloat16)
    bo2f = bo.astype(f64) + wo.astype(f64) @ bv.astype(f64)
    sca = np.array([[np.exp(np.float64(logit_scale[0])), logit_bias[0]]],
                   np.float32)
    return dict(
        cktb=np.ascontiguousarray(cktb),
        wvt=wvt, wot=wot,
        bo2=bo2f.reshape(1, DV).astype(np.float32),
        pjt=pjt, pjb=pb.reshape(1, DA).astype(np.float32),
        sca=sca,
    )


_NC_CACHE = {}


def kernel(video_x, audio_x, probe, wq, wk, wv, bq, bk, bv, wo, bo,
           ln_g, ln_b, proj_w, proj_b, logit_scale, logit_bias, T=None, H=None,
           W=None, **_unused):
    from concourse.bass_utils import run_bass_kernel_spmd

    video_x = np.asarray(video_x, np.float32)
    audio_x = np.asarray(audio_x, np.float32)
    params = host_fold(
        np.asarray(probe, np.float32), np.asarray(wq, np.float32),
        np.asarray(wk, np.float32), np.asarray(bq, np.float32),
        np.asarray(wv, np.float32), np.asarray(bv, np.float32),
        np.asarray(wo, np.float32), np.asarray(bo, np.float32),
        np.asarray(ln_g, np.float32), np.asarray(ln_b, np.float32),
        np.asarray(proj_w, np.float32), np.asarray(proj_b, np.float32),
        np.asarray(logit_scale, np.float32), np.asarray(logit_bias, np.float32),
    )
    if "nc" not in _NC_CACHE:
        _NC_CACHE["nc"] = build_nc()
    nc = _NC_CACHE["nc"]
    in_maps = []
    for b in range(B):
        m = dict(params)
        m["xv"] = np.ascontiguousarray(video_x[b])
        m["aud"] = np.ascontiguousarray(audio_x[b])
        in_maps.append(m)
    res = run_bass_kernel_spmd(nc, in_maps, core_ids=list(range(B)), trace=False)
    return np.stack([res.results[b]["out"] for b in range(B)], axis=0)

